# revision 24
# baseline (speedup 1.0000x reference)
"""BERT self-attention layer on 8 Trainium2 NeuronCores.

Sharding: tensor-parallel over heads. Each core owns 2 of the 16 heads
(a 128-wide slice of the QKV output dim). It computes q/k/v for its heads
over all 4096 tokens, runs attention for its (b, head) pairs, then an
AllToAll redistributes context from head-sharded to token-sharded layout.
Each core finishes with the full output projection + residual + LayerNorm
for its 512 tokens. Host concatenates the 8 token slices.

Attention math per (b, head): scores are computed transposed
(scoresT[key, q] on PSUM, keys on partitions) so softmax's exp runs on
ScalarE straight out of PSUM with the 1/sqrt(hd) scale fused. V is
augmented with a ones column so the ctx matmul also yields the softmax
denominator as row 64 (M=65); normalization multiplies by the
reciprocal broadcast across partitions (GpSimd partition_broadcast).
No max-subtraction is needed: scores/8 ~ N(0,1), exp is safe in fp32.
"""

import sys

sys.path.insert(0, "/opt/trn_rl_repo")

import numpy as np

import concourse.bass as bass
import concourse.mybir as mybir
import concourse.tile as tile
from concourse import bacc
from concourse.bass_utils import run_bass_kernel_spmd

FP32 = mybir.dt.float32
AF = mybir.ActivationFunctionType
ALU = mybir.AluOpType

B, S, H = 2, 2048, 1024
NH, HD = 16, 64
NC = 8                      # cores
HPC = NH // NC              # heads per core = 2
DPC = HPC * HD              # dims per core = 128
T = B * S                   # 4096 tokens
TPC = T // NC               # tokens per core in output = 512
SPB = S // NC               # per-b token slice per core = 256
P = 128
KCH = S // P                # key chunks per b = 16
HCH = H // P                # H contraction chunks = 8
QH = 1024                   # q half size
LN_EPS = 1e-05

_CACHE = {}


def build_kernel(dbg=False):
    nc = bacc.Bacc(
        "TRN2", target_bir_lowering=False, debug=False, num_devices=NC
    )

    # ---- kernel I/O (per-core) ----
    xT = nc.dram_tensor("xT", [H, T], FP32, kind="ExternalInput")
    wqT = nc.dram_tensor("wqT", [H, DPC], FP32, kind="ExternalInput")
    wkT = nc.dram_tensor("wkT", [H, DPC], FP32, kind="ExternalInput")
    wvT = nc.dram_tensor("wvT", [H, DPC], FP32, kind="ExternalInput")
    bq = nc.dram_tensor("bq", [DPC, 1], FP32, kind="ExternalInput")
    bk = nc.dram_tensor("bk", [DPC, 1], FP32, kind="ExternalInput")
    bvb = nc.dram_tensor("bvb", [P, DPC], FP32, kind="ExternalInput")
    wdT = nc.dram_tensor("wdT", [H, H], FP32, kind="ExternalInput")
    res = nc.dram_tensor("res", [TPC, H], FP32, kind="ExternalInput")
    gmb = nc.dram_tensor("gmb", [P, H], FP32, kind="ExternalInput")
    btb = nc.dram_tensor("btb", [P, H], FP32, kind="ExternalInput")
    out = nc.dram_tensor("out", [TPC, H], FP32, kind="ExternalOutput")
    if dbg:
        d_qT = nc.dram_tensor("d_qT", [P, T], FP32, kind="ExternalOutput")
        d_kT = nc.dram_tensor("d_kT", [P, T], FP32, kind="ExternalOutput")
        d_vA = nc.dram_tensor("d_vA", [P, T // P, HD + 1], FP32, kind="ExternalOutput")
        d_pr = nc.dram_tensor("d_pr", [P, QH], FP32, kind="ExternalOutput")
        d_cx = nc.dram_tensor("d_cx", [HD + 1, QH], FP32, kind="ExternalOutput")
        d_ct = nc.dram_tensor("d_ct", [HD, QH], FP32, kind="ExternalOutput")
        d_ci = nc.dram_tensor("d_ci", [P, HCH, SPB], FP32, kind="ExternalOutput")
        d_y = nc.dram_tensor("d_y", [P, H], FP32, kind="ExternalOutput")
        d_wd = nc.dram_tensor("d_wd", [P, HCH, H], FP32, kind="ExternalOutput")
        d_rs = nc.dram_tensor("d_rs", [P, H], FP32, kind="ExternalOutput")

    # partition-major views so the SBUF [128, chunks, n] destinations get
    # chunk c of the contraction dim at free slot c (elementwise DMA order)
    xT_c = xT.ap().rearrange("(c p) t -> p c t", p=P)       # [128,8,4096]
    wqT_c = wqT.ap().rearrange("(c p) m -> p c m", p=P)     # [128,8,128]
    wkT_c = wkT.ap().rearrange("(c p) m -> p c m", p=P)
    wvT_c = wvT.ap().rearrange("(c p) m -> p c m", p=P)
    wdT_c = wdT.ap().rearrange("(c p) o -> p c o", p=P)     # [128,8,1024]

    rg = [list(range(NC))]

    with tile.TileContext(nc) as tc:
        with (
            tc.tile_pool(name="dram", bufs=1, space="DRAM") as dpool,
            tc.tile_pool(name="singles", bufs=1) as sg,
            tc.tile_pool(name="probs", bufs=3) as ppool,
            tc.tile_pool(name="work", bufs=2) as wk,
            tc.tile_pool(name="ps_mm", bufs=2, space="PSUM") as ps_mm,
            tc.tile_pool(name="ps_sc", bufs=2, space="PSUM") as ps_sc,
            tc.tile_pool(name="ps_cx", bufs=1, space="PSUM") as ps_cx,
        ):
            # A2A buffers: shard j -> dest core j; [8, 128 dims, 256 toks]
            a2a_in0 = dpool.tile([NC, DPC, SPB], FP32)
            a2a_in1 = dpool.tile([NC, DPC, SPB], FP32)
            a2a_out0 = dpool.tile([NC, DPC, SPB], FP32)
            a2a_out1 = dpool.tile([NC, DPC, SPB], FP32)

            # ---- persistent SBUF tensors ----
            wq_sb = sg.tile([P, HCH, DPC], FP32)    # lhsT chunks [H128, 128]
            wk_sb = sg.tile([P, HCH, DPC], FP32)
            wv_sb = sg.tile([P, HCH, DPC], FP32)    # rhs chunks for v proj
            bq_sb = sg.tile([DPC, 1], FP32)
            bk_sb = sg.tile([DPC, 1], FP32)
            bvb_sb = sg.tile([P, DPC], FP32)
            qT_sb = sg.tile([P, T], FP32)           # [2 heads x 64, 4096]
            kT_sb = sg.tile([P, T], FP32)
            # v per head, token-major + ones col: [128, 32 chunks, 65]
            vA_sb = sg.tile([P, T // P, HD + 1], FP32)
            vB_sb = sg.tile([P, T // P, HD + 1], FP32)
            nc.vector.memset(vA_sb[:, :, HD : HD + 1], 1.0)
            nc.vector.memset(vB_sb[:, :, HD : HD + 1], 1.0)
            eps_sb = sg.tile([P, 1], FP32)
            nc.vector.memset(eps_sb[:], float(LN_EPS))

            nc.sync.dma_start(wq_sb[:], wqT_c)
            nc.sync.dma_start(wk_sb[:], wkT_c)
            nc.sync.dma_start(wv_sb[:], wvT_c)
            nc.sync.dma_start(bq_sb[:], bq.ap())
            nc.sync.dma_start(bk_sb[:], bk.ap())
            nc.sync.dma_start(bvb_sb[:], bvb.ap())

            # ================= QKV projections =================
            TB = 512  # token block
            xpool = tc.alloc_tile_pool(name="xblk", bufs=2)
            for blk in range(T // TB):
                t0 = blk * TB
                xb = xpool.tile([P, HCH, TB], FP32, tag="xb")
                nc.sync.dma_start(xb[:], xT_c[:, :, t0 : t0 + TB])  # [128,8,512]

                for w_sb, b_sb, dst in ((wq_sb, bq_sb, qT_sb), (wk_sb, bk_sb, kT_sb)):
                    pj = ps_mm.tile([P, TB], FP32, tag="mm")
                    for kc in range(HCH):
                        nc.tensor.matmul(
                            pj[:],
                            w_sb[:, kc, :],
                            xb[:, kc, :],
                            start=(kc == 0),
                            stop=(kc == HCH - 1),
                        )
                    nc.vector.tensor_scalar_add(
                        dst[:, t0 : t0 + TB], pj[:], b_sb[:]
                    )

                # v: token-major [128 toks, 128 dims] per 128-token tile
                for i in range(TB // P):
                    ch = blk * (TB // P) + i
                    pv = ps_mm.tile([P, DPC], FP32, tag="mm")
                    for kc in range(HCH):
                        nc.tensor.matmul(
                            pv[:],
                            xb[:, kc, i * P : (i + 1) * P],
                            wv_sb[:, kc, :],
                            start=(kc == 0),
                            stop=(kc == HCH - 1),
                        )
                    nc.vector.tensor_tensor(
                        vA_sb[:, ch, 0:HD], pv[:, 0:HD], bvb_sb[:, 0:HD], ALU.add
                    )
                    nc.vector.tensor_tensor(
                        vB_sb[:, ch, 0:HD], pv[:, HD:DPC], bvb_sb[:, HD:DPC], ALU.add
                    )

            # ================= attention =================
            def attn_b(b):
                a2a_in = a2a_in0 if b == 0 else a2a_in1
                tb0 = b * S
                for h in range(HPC):
                    r0 = h * HD  # partition offset of this head in qT/kT
                    v_sb = vA_sb if h == 0 else vB_sb
                    for qh in range(S // QH):
                        q0 = tb0 + qh * QH
                        cx = ps_cx.tile([HD + 1, QH], FP32, tag="cx")
                        for kc in range(KCH):
                            kt0 = tb0 + kc * P
                            sc = ps_sc.tile([P, QH], FP32, tag="sc")
                            for half in range(QH // 512):
                                nc.tensor.matmul(
                                    sc[:, half * 512 : (half + 1) * 512],
                                    kT_sb[r0 : r0 + HD, kt0 : kt0 + P],
                                    qT_sb[
                                        r0 : r0 + HD,
                                        q0 + half * 512 : q0 + (half + 1) * 512,
                                    ],
                                    start=True,
                                    stop=True,
                                )
                            pr = ppool.tile([P, QH], FP32, tag="pr")
                            nc.scalar.activation(
                                pr[:], sc[:], AF.Exp, scale=float(1.0 / np.sqrt(HD))
                            )
                            if dbg and b == 0 and h == 0 and qh == 0 and kc == 0:
                                nc.sync.dma_start(d_pr.ap(), pr[:])
                            vch = (tb0 // P) + kc
                            for half in range(QH // 512):
                                nc.tensor.matmul(
                                    cx[:, half * 512 : (half + 1) * 512],
                                    v_sb[:, vch, :],
                                    pr[:, half * 512 : (half + 1) * 512],
                                    start=(kc == 0),
                                    stop=(kc == KCH - 1),
                                )
                        if dbg and b == 0 and h == 0 and qh == 0:
                            cxc = wk.tile([HD + 1, QH], FP32, tag="cxc")
                            nc.vector.tensor_copy(cxc[:], cx[:])
                            nc.sync.dma_start(d_cx.ap(), cxc[:])
                        # normalize: recip of denom row, broadcast, multiply
                        rc = wk.tile([1, QH], FP32, tag="rc")
                        nc.vector.reciprocal(rc[:], cx[HD : HD + 1, :])
                        rcb = wk.tile([HD, QH], FP32, tag="rcb")
                        nc.gpsimd.partition_broadcast(rcb[:], rc[:], channels=HD)
                        ct = wk.tile([HD, QH], FP32, tag="ct")
                        nc.vector.tensor_tensor(ct[:], cx[0:HD, :], rcb[:], ALU.mult)
                        if dbg and b == 0 and h == 0 and qh == 0:
                            nc.sync.dma_start(d_ct.ap(), ct[:])
                        # scatter to A2A input: QH spans QH//SPB shards
                        sh0 = qh * (QH // SPB)
                        nc.sync.dma_start(
                            a2a_in[sh0 : sh0 + QH // SPB, r0 : r0 + HD, :].rearrange(
                                "j p q -> p j q"
                            ),
                            ct.rearrange("p (j q) -> p j q", q=SPB),
                        )

            xpool.release()
            if dbg:
                nc.sync.dma_start(d_qT.ap(), qT_sb[:])
                nc.sync.dma_start(d_kT.ap(), kT_sb[:])
                nc.sync.dma_start(d_vA.ap(), vA_sb[:])

            # ============ output stage for one b ============
            ost = tc.alloc_tile_pool(name="ostage", bufs=1)
            wd_sb = sg.tile([P, HCH, H], FP32)
            nc.sync.dma_start(wd_sb[:], wdT_c)
            gmb_sb = sg.tile([P, H], FP32)
            btb_sb = sg.tile([P, H], FP32)
            nc.sync.dma_start(gmb_sb[:], gmb.ap())
            nc.sync.dma_start(btb_sb[:], btb.ap())

            def outproj_b(b, y_tiles):
                a2a_out = a2a_out0 if b == 0 else a2a_out1
                ci = ost.tile([P, HCH, SPB], FP32, tag="ci")
                nc.sync.dma_start(ci[:], a2a_out.rearrange("j p q -> p j q"))
                if dbg and b == 0:
                    nc.sync.dma_start(d_ci.ap(), ci[:])
                    nc.sync.dma_start(d_wd.ap(), wd_sb[:])
                for tt in range(SPB // P):
                    rrow = b * SPB + tt * P
                    rs = ost.tile([P, H], FP32, tag="rs")
                    nc.sync.dma_start(rs[:], res.ap()[rrow : rrow + P, :])
                    y = sg.tile([P, H], FP32, name=f"y_{b}_{tt}", tag=f"y{b}{tt}")
                    for nb in range(H // 512):
                        po = ps_mm.tile([P, 512], FP32, tag="mm")
                        for kc in range(HCH):
                            nc.tensor.matmul(
                                po[:],
                                ci[:, kc, tt * P : (tt + 1) * P],
                                wd_sb[:, kc, nb * 512 : (nb + 1) * 512],
                                start=(kc == 0),
                                stop=(kc == HCH - 1),
                            )
                        nc.vector.tensor_tensor(
                            y[:, nb * 512 : (nb + 1) * 512],
                            po[:],
                            rs[:, nb * 512 : (nb + 1) * 512],
                            ALU.add,
                        )
                    if dbg and b == 0 and tt == 0:
                        nc.sync.dma_start(d_y.ap(), y[:])
                        nc.sync.dma_start(d_rs.ap(), rs[:])
                    y_tiles.append((b, tt, y))

            def layernorm(b, tt, y):
                ms = wk.tile([P, 1], FP32, tag="ms")
                nc.vector.reduce_sum(ms[:], y[:], axis=mybir.AxisListType.X)
                mean = wk.tile([P, 1], FP32, tag="mean")
                nc.vector.tensor_scalar_mul(mean[:], ms[:], 1.0 / H)
                cen = ost.tile([P, H], FP32, tag="cen")
                nc.vector.tensor_scalar(
                    cen[:], y[:], mean[:], None, ALU.subtract
                )
                vs = wk.tile([P, 1], FP32, tag="vs")
                # square's output is unused (only the free-dim accumulation);
                # dump it over y, which is dead after centering
                nc.scalar.activation(y[:], cen[:], AF.Square, accum_out=vs[:])
                # rstd = exp(-0.5 * ln(var + eps)); Ln/Exp share a table set
                lnv = wk.tile([P, 1], FP32, tag="lnv")
                nc.scalar.activation(
                    lnv[:], vs[:], AF.Ln, scale=float(1.0 / H), bias=eps_sb[:]
                )
                nl = wk.tile([P, 1], FP32, tag="nl")
                nc.vector.tensor_scalar_mul(nl[:], lnv[:], -0.5)
                rstd = wk.tile([P, 1], FP32, tag="rstd")
                nc.scalar.activation(rstd[:], nl[:], AF.Exp)
                o1 = ost.tile([P, H], FP32, tag="o1")
                nc.vector.tensor_scalar_mul(o1[:], cen[:], rstd[:])
                nc.vector.tensor_tensor(o1[:], o1[:], gmb_sb[:], ALU.mult)
                nc.vector.tensor_tensor(o1[:], o1[:], btb_sb[:], ALU.add)
                orow = b * SPB + tt * P
                nc.sync.dma_start(out.ap()[orow : orow + P, :], o1[:])

            y_tiles = []
            attn_b(0)
            tc.strict_bb_all_engine_barrier()
            nc.gpsimd.collective_compute(
                "AllToAll",
                ALU.bypass,
                replica_groups=rg,
                ins=[a2a_in0[:].opt()],
                outs=[a2a_out0[:].opt()],
            )
            tc.strict_bb_all_engine_barrier()
            outproj_b(0, y_tiles)
            attn_b(1)
            tc.strict_bb_all_engine_barrier()
            nc.gpsimd.collective_compute(
                "AllToAll",
                ALU.bypass,
                replica_groups=rg,
                ins=[a2a_in1[:].opt()],
                outs=[a2a_out1[:].opt()],
            )
            tc.strict_bb_all_engine_barrier()
            outproj_b(1, y_tiles)
            for b, tt, y in y_tiles:
                layernorm(b, tt, y)
            ost.release()

    nc.compile()
    return nc


def prep_inputs(hidden_states, Wq, bq, Wk, bk, Wv, bv, Wd, bd, ln_gamma, ln_beta):
    x = np.ascontiguousarray(np.asarray(hidden_states, np.float32).reshape(T, H))
    xT = np.ascontiguousarray(x.T)
    wdT = np.ascontiguousarray(np.asarray(Wd, np.float32).T)
    gmb = np.ascontiguousarray(np.broadcast_to(ln_gamma, (P, H)), dtype=np.float32)
    btb = np.ascontiguousarray(np.broadcast_to(ln_beta, (P, H)), dtype=np.float32)
    in_maps = []
    for c in range(NC):
        d0 = c * DPC
        # core c's output tokens: b0 s in [c*SPB, (c+1)*SPB), b1 same
        rows = np.concatenate(
            [np.arange(c * SPB, (c + 1) * SPB), S + np.arange(c * SPB, (c + 1) * SPB)]
        )
        res = x[rows] + np.asarray(bd, np.float32)[None, :]
        in_maps.append(
            {
                "xT": xT,
                "wqT": np.ascontiguousarray(Wq[d0 : d0 + DPC, :].T),
                "wkT": np.ascontiguousarray(Wk[d0 : d0 + DPC, :].T),
                "wvT": np.ascontiguousarray(Wv[d0 : d0 + DPC, :].T),
                "bq": np.ascontiguousarray(bq[d0 : d0 + DPC, None]),
                "bk": np.ascontiguousarray(bk[d0 : d0 + DPC, None]),
                "bvb": np.ascontiguousarray(
                    np.broadcast_to(bv[d0 : d0 + DPC], (P, DPC)), dtype=np.float32
                ),
                "wdT": wdT,
                "res": np.ascontiguousarray(res),
                "gmb": gmb,
                "btb": btb,
            }
        )
    return in_maps


def kernel(**inputs):
    if "nc" not in _CACHE:
        _CACHE["nc"] = build_kernel()
    nc = _CACHE["nc"]
    in_maps = prep_inputs(**inputs)
    r = run_bass_kernel_spmd(nc, in_maps, core_ids=list(range(NC)))
    out = np.empty((B, S, H), np.float32)
    for c in range(NC):
        o = r.results[c]["out"]
        out[0, c * SPB : (c + 1) * SPB] = o[:SPB]
        out[1, c * SPB : (c + 1) * SPB] = o[SPB:]
    return out


if __name__ == "__main__":
    import reference

    inputs = {k: np.asarray(v) for k, v in reference.setup_inputs().items()}
    got = kernel(**inputs)
    exp = np.asarray(reference.reference(**inputs))
    err = np.abs(got - exp)
    rel = np.linalg.norm(got - exp) / np.linalg.norm(exp)
    print("max abs err:", err.max(), "rel:", rel)
